# revision 58
# baseline (speedup 1.0000x reference)
"""Trainium2 kernel for the FEM kinematic (strain) layer.

Reference computation:
    disp = inputs[:, elem_nodes]                      # [B, E, 8, 2]
    dd   = einsum('egkl,bekn->begnl', shpdx, disp)    # [B, E, 9, 2, 2]
    out  = stack([dd[...,0,0], dd[...,1,1],
                  0.5*(dd[...,0,1] + dd[...,1,0])])   # [B, E*9, 3]

Sharding: elements split across 8 NeuronCores.  The host resolves the
element->node indirection and ships each core bf16 element-major blocks:
  S' = (s0, s1, (s0+s1)/2) per (e,g,k)   and   D' = (u, v, u+v) per (e,b,k).
The device computes, per (b,e,g), the three 8-term dot products
  P0 = s0.u, P1 = s1.v, P2 = 0.5*(s0+s1).(u+v)
via one bf16 tensor_tensor multiply (DVE 2x mode) + a pairwise add tree,
then combines  xx = P0, yy = P1, xy = P2 - 0.5*P0 - 0.5*P1.
Tree/combine work is split between the Vector and GpSimd engines.
"""

import sys
import numpy as np

sys.path.insert(0, "/opt/trn_rl_repo")

import concourse.bass as bass
import concourse.bacc as bacc
import concourse.mybir as mybir
import concourse.tile as tile
from concourse.bass_utils import run_bass_kernel_spmd

import ml_dtypes

BF16 = ml_dtypes.bfloat16

B = 4
N_NODES = 1_000_000
N_ELEM = 500_000
N_GP = 9
N_EN = 8
N_CORES = 8

E_CORE = N_ELEM // N_CORES            # 62500 elements per core
P = 128                               # SBUF partitions
C = 32                                # elements per partition per full chunk
IO_BUFS = 3
MID_BUFS = 1                          # per-b tile tags already pipeline
T_BUFS = 1
POOL_L1 = 0                           # L1 all on DVE; GpSimd takes L2+xy
XY_ACT = 1                            # xy fixup via ScalarE halves + GpSimd adds
DVE_L2 = 1                            # one L2 on DVE, rest GpSimd
CHUNK = P * C                         # 4864 elements per full chunk
N_FULL = E_CORE // CHUNK              # 12 full chunks
REM = E_CORE - N_FULL * CHUNK         # 4132 leftover elements
C_LAST = -(-REM // P)                 # 33: last-chunk elements/partition
N_CHUNKS = N_FULL + (1 if REM else 0)
E_PAD = N_FULL * CHUNK + (P * C_LAST if REM else 0)   # 62592
CS = [C] * N_FULL + ([C_LAST] if REM else [])
OFFS = [i * CHUNK for i in range(N_FULL)] + ([N_FULL * CHUNK] if REM else [])

S_EL = N_GP * 3 * N_EN                # 216: per element (g, t, k)
D_EL = B * 3 * N_EN                   # 96:  per element (b, t, k)
O_EL = B * N_GP * 3                   # 108: per element (b, g, t)

_compiled = None


def _build_program():
    nc = bacc.Bacc("TRN2", target_bir_lowering=False, debug=False)
    f32 = mybir.dt.float32
    bf16 = mybir.dt.bfloat16

    s_d = nc.dram_tensor("sp", [E_PAD, S_EL], bf16, kind="ExternalInput").ap()
    d_d = nc.dram_tensor("dp", [E_PAD, D_EL], bf16, kind="ExternalInput").ap()
    o_d = nc.dram_tensor("out", [B, E_PAD * N_GP, 3], bf16, kind="ExternalOutput").ap()

    def s_view(i):
        ci = CS[i]
        return s_d[OFFS[i]:OFFS[i] + P * ci].rearrange(
            "(p c) f -> p (c f)", p=P, c=ci)

    def d_view(i):
        ci = CS[i]
        return d_d[OFFS[i]:OFFS[i] + P * ci].rearrange(
            "(p c) f -> p (c f)", p=P, c=ci)

    def o_view(i):
        ci = CS[i]
        return o_d[:, OFFS[i] * N_GP:(OFFS[i] + P * ci) * N_GP].rearrange(
            "b (p c g) t -> p b (c g t)", p=P, c=ci, g=N_GP)

    with tile.TileContext(nc) as tc:
        with (
            tc.tile_pool(name="io", bufs=IO_BUFS) as io_pool,
            tc.tile_pool(name="mid", bufs=MID_BUFS) as mid_pool,
            tc.tile_pool(name="tp", bufs=T_BUFS) as t_pool,
            tc.tile_pool(name="u2p", bufs=2) as u2_pool,
        ):
            def issue_front(i):
                """DMAs + products + tree for chunk i; returns tail state."""
                ci = CS[i]
                S = io_pool.tile([P, C * S_EL], bf16, tag="S")
                D = io_pool.tile([P, C * D_EL], bf16, tag="D")
                nc.sync.dma_start(out=S[:, :ci * S_EL], in_=s_view(i))
                nc.sync.dma_start(out=D[:, :ci * D_EL], in_=d_view(i))

                Sv = S[:, :ci * S_EL].rearrange(
                    "p (c g t k) -> p c g t k", c=ci, g=N_GP, t=3)
                Dv = D[:, :ci * D_EL].rearrange(
                    "p (c b t k) -> p c b t k", c=ci, b=B, t=3)

                Tbs, U2bs = [], []
                U1bs = []
                def mk_prod(b):
                    Tb = t_pool.tile([P, C * 216], bf16, tag=f"T{b}")
                    Tbs.append(Tb)
                    db = Dv[:, :, b]
                    dbg = db[:, :, None, :, :].to_broadcast([P, ci, N_GP, 3, N_EN])
                    Tbv = Tb[:, :ci * 216].rearrange("p (c g t k) -> p c g t k", c=ci, g=N_GP, t=3)
                    nc.vector.tensor_tensor(
                        out=Tbv, in0=Sv, in1=dbg, op=mybir.AluOpType.mult
                    )
                def mk_l1(b):
                    U1b = mid_pool.tile([P, C * 108], bf16, tag=f"U1{b}")
                    U1bs.append(U1b)
                    Tbv = Tbs[b][:, :ci * 216].rearrange("p (c g t k) -> p c g t k", c=ci, g=N_GP, t=3)
                    U1bv = U1b[:, :ci * 108].rearrange("p (c g t k) -> p c g t k", c=ci, g=N_GP, t=3)
                    eng = nc.gpsimd if b < POOL_L1 else nc.vector
                    eng.tensor_tensor(
                        out=U1bv,
                        in0=Tbv[:, :, :, :, 0:4],
                        in1=Tbv[:, :, :, :, 4:8],
                        op=mybir.AluOpType.add,
                    )
                for b in range(B):
                    mk_prod(b)
                for b in range(B):
                    mk_l1(b)
                for b in range(B):
                    U2b = u2_pool.tile([P, C * 54], bf16, tag=f"U2{b}")
                    U2bs.append(U2b)
                    U1bv = U1bs[b][:, :ci * 108].rearrange("p (c g t k) -> p c g t k", c=ci, g=N_GP, t=3)
                    U2bv = U2b[:, :ci * 54].rearrange("p (c g t k) -> p c g t k", c=ci, g=N_GP, t=3)
                    eng2 = nc.vector if b < DVE_L2 else nc.gpsimd
                    eng2.tensor_tensor(
                        out=U2bv,
                        in0=U1bv[:, :, :, :, 0:2],
                        in1=U1bv[:, :, :, :, 2:4],
                        op=mybir.AluOpType.add,
                    )
                return ci, U2bs

            def issue_tail(i, ci, U2bs):
                """Strain combine + xy fixup + output DMA for chunk i."""
                O = io_pool.tile([P, B * C * 27], bf16, tag="O")
                Ocb = O[:, :B * ci * 27].rearrange(
                    "p (b c g t) -> p c b g t", b=B, c=ci, g=N_GP)
                TMP = u2_pool.tile([P, C * B * N_GP], bf16, tag="TMP")
                TMPv = TMP[:, :ci * B * N_GP].rearrange(
                    "p (c b g) -> p c b g", c=ci, b=B)
                TMP2 = u2_pool.tile([P, C * B * N_GP], bf16, tag="TMP2")
                TMP2v = TMP2[:, :ci * B * N_GP].rearrange(
                    "p (c b g) -> p c b g", c=ci, b=B)
                xx_o = Ocb[:, :, :, :, 0]
                yy_o = Ocb[:, :, :, :, 1]
                xy_o = Ocb[:, :, :, :, 2]
                for b in range(B):
                    U2bv = U2bs[b][:, :ci * 54].rearrange(
                        "p (c g t k) -> p c g t k", c=ci, g=N_GP, t=3
                    )
                    # one add writes (xx, yy, P2) straight into O's three
                    # strain slots; the xy slot is then fixed up in place:
                    #   xy = P2 - 0.5*xx - 0.5*yy
                    engc = nc.gpsimd
                    engc.tensor_tensor(
                        out=Ocb[:, :, b], in0=U2bv[:, :, :, :, 0],
                        in1=U2bv[:, :, :, :, 1], op=mybir.AluOpType.add,
                    )
                    if XY_ACT:
                        # xy fixup without touching DVE: ScalarE halves,
                        # GpSimd adds
                        nc.scalar.activation(
                            out=TMPv[:, :, b], in_=xx_o[:, :, b],
                            func=mybir.ActivationFunctionType.Copy, scale=-0.5,
                        )
                        nc.scalar.activation(
                            out=TMP2v[:, :, b], in_=yy_o[:, :, b],
                            func=mybir.ActivationFunctionType.Copy, scale=-0.5,
                        )
                        engx = nc.gpsimd
                        engx.tensor_tensor(
                            out=TMPv[:, :, b], in0=TMPv[:, :, b],
                            in1=TMP2v[:, :, b], op=mybir.AluOpType.add,
                        )
                        engx.tensor_tensor(
                            out=xy_o[:, :, b], in0=TMPv[:, :, b],
                            in1=xy_o[:, :, b], op=mybir.AluOpType.add,
                        )
                    else:
                        nc.vector.scalar_tensor_tensor(
                            out=TMPv[:, :, b], in0=xx_o[:, :, b], scalar=-0.5,
                            in1=xy_o[:, :, b],
                            op0=mybir.AluOpType.mult, op1=mybir.AluOpType.add,
                        )
                        nc.vector.scalar_tensor_tensor(
                            out=xy_o[:, :, b], in0=yy_o[:, :, b], scalar=-0.5,
                            in1=TMPv[:, :, b],
                            op0=mybir.AluOpType.mult, op1=mybir.AluOpType.add,
                        )
                Ob = O[:, :B * ci * 27].rearrange("p (b f) -> p b f", b=B)
                # split per b-pair: first half ships while b2/b3 still combine
                ov = o_view(i)
                nc.sync.dma_start(out=ov[:, 0:2], in_=Ob[:, 0:2])
                nc.sync.dma_start(out=ov[:, 2:4], in_=Ob[:, 2:4])

            # software-pipelined issue: chunk i's tail is issued after chunk
            # i+1's front so the DVE FIFO never head-blocks on Pool's L2
            # software-pipelined issue: chunk i's tail (combine + xy fixup +
            # output DMA) is issued after chunk i+1's front, so the DVE FIFO
            # never head-blocks on GpSimd's L2 results
            # software-pipelined issue: chunk i's tail (combine + xy fixup +
            # output DMA) is issued after chunk i+1's front, so the DVE FIFO
            # never head-blocks on GpSimd's L2 results
            prev = None
            for i in range(N_CHUNKS):
                ci_u2 = issue_front(i)
                if prev is not None:
                    issue_tail(i - 1, *prev)
                prev = ci_u2
            issue_tail(N_CHUNKS - 1, *prev)

    nc.compile()
    return nc


def _get_program():
    global _compiled
    if _compiled is None:
        _compiled = _build_program()
    return _compiled


def _marshal_core(inputs, shpdx, elem_nodes, c):
    """Build the per-core bf16 S'/D' blocks."""
    sl = slice(c * E_CORE, (c + 1) * E_CORE)
    en = elem_nodes[sl]                                   # [E, 8]
    disp = inputs[:, en]                                  # [B, E, 8, 2]
    u = disp[..., 0]                                      # [B, E, 8]
    v = disp[..., 1]
    w = u + v
    # D'[e, b, t, k] with t = (u, v, w)
    dstk = np.stack([u, v, w], axis=2)                    # [B, E, 3, 8]
    dstk = dstk.transpose(1, 0, 2, 3)                     # [E, B, 3, 8]
    dpad = np.zeros((E_PAD, D_EL), BF16)
    dpad[:E_CORE] = dstk.reshape(E_CORE, D_EL).astype(BF16)

    sx = shpdx[sl]                                        # [E, 9, 8, 2]
    s0 = sx[..., 0]
    s1 = sx[..., 1]
    a = 0.5 * (s0 + s1)
    sstk = np.stack([s0, s1, a], axis=2)                  # [E, 9, 3, 8]
    spad = np.zeros((E_PAD, S_EL), BF16)
    spad[:E_CORE] = sstk.reshape(E_CORE, S_EL).astype(BF16)
    return {"sp": spad, "dp": dpad}


def kernel(inputs, shpdx, elem_nodes, _want_trace=False):
    nc = _get_program()

    in_maps = [
        _marshal_core(inputs, shpdx, elem_nodes, c) for c in range(N_CORES)
    ]

    core_ids = list(range(N_CORES))
    res = run_bass_kernel_spmd(nc, in_maps, core_ids, trace=_want_trace)

    outs = []
    for c in range(N_CORES):
        o = res.results[c]["out"]                         # [B, E_PAD*9, 3] bf16
        outs.append(np.asarray(o[:, :E_CORE * N_GP, :], np.float32))
    full = np.concatenate(outs, axis=1)                   # [B, N_ELEM*9, 3]
    if _want_trace:
        return full, res
    return full


# revision 60
# speedup vs baseline: 1.0043x; 1.0043x over previous
"""Trainium2 kernel for the FEM kinematic (strain) layer.

Reference computation:
    disp = inputs[:, elem_nodes]                      # [B, E, 8, 2]
    dd   = einsum('egkl,bekn->begnl', shpdx, disp)    # [B, E, 9, 2, 2]
    out  = stack([dd[...,0,0], dd[...,1,1],
                  0.5*(dd[...,0,1] + dd[...,1,0])])   # [B, E*9, 3]

Sharding: elements split across 8 NeuronCores.  The host resolves the
element->node indirection and ships each core bf16 element-major blocks:
  S' = (s0, s1, (s0+s1)/2) per (e,g,k)   and   D' = (u, v, u+v) per (e,b,k).
The device computes, per (b,e,g), the three 8-term dot products
  P0 = s0.u, P1 = s1.v, P2 = 0.5*(s0+s1).(u+v)
via one bf16 tensor_tensor multiply (DVE 2x mode) + a pairwise add tree,
then combines  xx = P0, yy = P1, xy = P2 - 0.5*P0 - 0.5*P1.
Tree/combine work is split between the Vector and GpSimd engines.
"""

import sys
import numpy as np

sys.path.insert(0, "/opt/trn_rl_repo")

import concourse.bass as bass
import concourse.bacc as bacc
import concourse.mybir as mybir
import concourse.tile as tile
from concourse.bass_utils import run_bass_kernel_spmd

import ml_dtypes

BF16 = ml_dtypes.bfloat16

B = 4
N_NODES = 1_000_000
N_ELEM = 500_000
N_GP = 9
N_EN = 8
N_CORES = 8

E_CORE = N_ELEM // N_CORES            # 62500 elements per core
P = 128                               # SBUF partitions
C = 32                                # elements per partition per full chunk
IO_BUFS = 3
MID_BUFS = 1                          # per-b tile tags already pipeline
T_BUFS = 1
POOL_L1 = 0                           # L1 all on DVE; GpSimd takes L2+xy
XY_ACT = 1                            # xy fixup via ScalarE halves + GpSimd adds
DVE_L2 = 1                            # one L2 on DVE, rest GpSimd
CHUNK = P * C                         # 4864 elements per full chunk
N_FULL = E_CORE // CHUNK              # 12 full chunks
REM = E_CORE - N_FULL * CHUNK         # 4132 leftover elements
C_LAST = -(-REM // P)                 # 33: last-chunk elements/partition
N_CHUNKS = N_FULL + (1 if REM else 0)
E_PAD = N_FULL * CHUNK + (P * C_LAST if REM else 0)   # 62592
CS = [C] * N_FULL + ([C_LAST] if REM else [])
OFFS = [i * CHUNK for i in range(N_FULL)] + ([N_FULL * CHUNK] if REM else [])

S_EL = N_GP * 3 * N_EN                # 216: per element (g, t, k)
D_EL = B * 3 * N_EN                   # 96:  per element (b, t, k)
O_EL = B * N_GP * 3                   # 108: per element (b, g, t)

_compiled = None


def _build_program():
    nc = bacc.Bacc("TRN2", target_bir_lowering=False, debug=False)
    f32 = mybir.dt.float32
    bf16 = mybir.dt.bfloat16

    s_d = nc.dram_tensor("sp", [E_PAD, S_EL], bf16, kind="ExternalInput").ap()
    d_d = nc.dram_tensor("dp", [E_PAD, D_EL], bf16, kind="ExternalInput").ap()
    o_d = nc.dram_tensor("out", [B, E_PAD * N_GP, 3], bf16, kind="ExternalOutput").ap()

    def s_view(i):
        ci = CS[i]
        return s_d[OFFS[i]:OFFS[i] + P * ci].rearrange(
            "(p c) f -> p (c f)", p=P, c=ci)

    def d_view(i):
        ci = CS[i]
        return d_d[OFFS[i]:OFFS[i] + P * ci].rearrange(
            "(p c) f -> p (c f)", p=P, c=ci)

    def o_view(i):
        ci = CS[i]
        return o_d[:, OFFS[i] * N_GP:(OFFS[i] + P * ci) * N_GP].rearrange(
            "b (p c g) t -> p b (c g t)", p=P, c=ci, g=N_GP)

    with tile.TileContext(nc) as tc:
        with (
            tc.tile_pool(name="io", bufs=IO_BUFS) as io_pool,
            tc.tile_pool(name="mid", bufs=MID_BUFS) as mid_pool,
            tc.tile_pool(name="tp", bufs=T_BUFS) as t_pool,
            tc.tile_pool(name="u2p", bufs=2) as u2_pool,
        ):
            def issue_front(i):
                """DMAs + products + tree for chunk i; returns tail state."""
                ci = CS[i]
                S = io_pool.tile([P, C * S_EL], bf16, tag="S")
                D = io_pool.tile([P, C * D_EL], bf16, tag="D")
                nc.sync.dma_start(out=S[:, :ci * S_EL], in_=s_view(i))
                nc.sync.dma_start(out=D[:, :ci * D_EL], in_=d_view(i))

                Sv = S[:, :ci * S_EL].rearrange(
                    "p (c g t k) -> p c g t k", c=ci, g=N_GP, t=3)
                Dv = D[:, :ci * D_EL].rearrange(
                    "p (c b t k) -> p c b t k", c=ci, b=B, t=3)

                Tbs, U2bs = [], []
                U1bs = []
                def mk_prod(b):
                    Tb = t_pool.tile([P, C * 216], bf16, tag=f"T{b}")
                    Tbs.append(Tb)
                    db = Dv[:, :, b]
                    dbg = db[:, :, None, :, :].to_broadcast([P, ci, N_GP, 3, N_EN])
                    Tbv = Tb[:, :ci * 216].rearrange("p (c g t k) -> p c g t k", c=ci, g=N_GP, t=3)
                    nc.vector.tensor_tensor(
                        out=Tbv, in0=Sv, in1=dbg, op=mybir.AluOpType.mult
                    )
                def mk_l1(b):
                    U1b = mid_pool.tile([P, C * 108], bf16, tag=f"U1{b}")
                    U1bs.append(U1b)
                    Tbv = Tbs[b][:, :ci * 216].rearrange("p (c g t k) -> p c g t k", c=ci, g=N_GP, t=3)
                    U1bv = U1b[:, :ci * 108].rearrange("p (c g t k) -> p c g t k", c=ci, g=N_GP, t=3)
                    eng = nc.gpsimd if b < POOL_L1 else nc.vector
                    eng.tensor_tensor(
                        out=U1bv,
                        in0=Tbv[:, :, :, :, 0:4],
                        in1=Tbv[:, :, :, :, 4:8],
                        op=mybir.AluOpType.add,
                    )
                for b in range(B):
                    mk_prod(b)
                for b in range(B):
                    mk_l1(b)
                for b in range(B):
                    U2b = u2_pool.tile([P, C * 54], bf16, tag=f"U2{b}")
                    U2bs.append(U2b)
                    U1bv = U1bs[b][:, :ci * 108].rearrange("p (c g t k) -> p c g t k", c=ci, g=N_GP, t=3)
                    U2bv = U2b[:, :ci * 54].rearrange("p (c g t k) -> p c g t k", c=ci, g=N_GP, t=3)
                    eng2 = nc.vector if b < DVE_L2 else nc.gpsimd
                    eng2.tensor_tensor(
                        out=U2bv,
                        in0=U1bv[:, :, :, :, 0:2],
                        in1=U1bv[:, :, :, :, 2:4],
                        op=mybir.AluOpType.add,
                    )
                return ci, U2bs

            def issue_tail(i, ci, U2bs, last=False):
                """Strain combine + xy fixup + output DMA for chunk i."""
                O = io_pool.tile([P, B * C * 27], bf16, tag="O")
                Ocb = O[:, :B * ci * 27].rearrange(
                    "p (b c g t) -> p c b g t", b=B, c=ci, g=N_GP)
                TMP = u2_pool.tile([P, C * B * N_GP], bf16, tag="TMP")
                TMPv = TMP[:, :ci * B * N_GP].rearrange(
                    "p (c b g) -> p c b g", c=ci, b=B)
                if not last:
                    TMP2 = u2_pool.tile([P, C * B * N_GP], bf16, tag="TMP2")
                    TMP2v = TMP2[:, :ci * B * N_GP].rearrange(
                        "p (c b g) -> p c b g", c=ci, b=B)
                xx_o = Ocb[:, :, :, :, 0]
                yy_o = Ocb[:, :, :, :, 1]
                xy_o = Ocb[:, :, :, :, 2]
                for b in range(B):
                    U2bv = U2bs[b][:, :ci * 54].rearrange(
                        "p (c g t k) -> p c g t k", c=ci, g=N_GP, t=3
                    )
                    # one add writes (xx, yy, P2) straight into O's three
                    # strain slots; the xy slot is then fixed up in place:
                    #   xy = P2 - 0.5*xx - 0.5*yy
                    engc = nc.gpsimd
                    engc.tensor_tensor(
                        out=Ocb[:, :, b], in0=U2bv[:, :, :, :, 0],
                        in1=U2bv[:, :, :, :, 1], op=mybir.AluOpType.add,
                    )
                    if XY_ACT and not last:
                        # xy fixup without touching DVE: ScalarE halves,
                        # GpSimd adds
                        nc.scalar.activation(
                            out=TMPv[:, :, b], in_=xx_o[:, :, b],
                            func=mybir.ActivationFunctionType.Copy, scale=-0.5,
                        )
                        nc.scalar.activation(
                            out=TMP2v[:, :, b], in_=yy_o[:, :, b],
                            func=mybir.ActivationFunctionType.Copy, scale=-0.5,
                        )
                        engx = nc.gpsimd
                        engx.tensor_tensor(
                            out=TMPv[:, :, b], in0=TMPv[:, :, b],
                            in1=TMP2v[:, :, b], op=mybir.AluOpType.add,
                        )
                        engx.tensor_tensor(
                            out=xy_o[:, :, b], in0=TMPv[:, :, b],
                            in1=xy_o[:, :, b], op=mybir.AluOpType.add,
                        )
                    else:
                        # short serial chain for the final drain
                        nc.vector.scalar_tensor_tensor(
                            out=TMPv[:, :, b], in0=xx_o[:, :, b], scalar=-0.5,
                            in1=xy_o[:, :, b],
                            op0=mybir.AluOpType.mult, op1=mybir.AluOpType.add,
                        )
                        nc.vector.scalar_tensor_tensor(
                            out=xy_o[:, :, b], in0=yy_o[:, :, b], scalar=-0.5,
                            in1=TMPv[:, :, b],
                            op0=mybir.AluOpType.mult, op1=mybir.AluOpType.add,
                        )
                Ob = O[:, :B * ci * 27].rearrange("p (b f) -> p b f", b=B)
                # split per b-pair: first half ships while b2/b3 still combine
                ov = o_view(i)
                nc.sync.dma_start(out=ov[:, 0:2], in_=Ob[:, 0:2])
                nc.sync.dma_start(out=ov[:, 2:4], in_=Ob[:, 2:4])

            # software-pipelined issue: chunk i's tail is issued after chunk
            # i+1's front so the DVE FIFO never head-blocks on Pool's L2
            # software-pipelined issue: chunk i's tail (combine + xy fixup +
            # output DMA) is issued after chunk i+1's front, so the DVE FIFO
            # never head-blocks on GpSimd's L2 results
            # software-pipelined issue: chunk i's tail (combine + xy fixup +
            # output DMA) is issued after chunk i+1's front, so the DVE FIFO
            # never head-blocks on GpSimd's L2 results
            prev = None
            for i in range(N_CHUNKS):
                ci_u2 = issue_front(i)
                if prev is not None:
                    issue_tail(i - 1, *prev)
                prev = ci_u2
            issue_tail(N_CHUNKS - 1, *prev, last=True)

    nc.compile()
    return nc


def _get_program():
    global _compiled
    if _compiled is None:
        _compiled = _build_program()
    return _compiled


def _marshal_core(inputs, shpdx, elem_nodes, c):
    """Build the per-core bf16 S'/D' blocks."""
    sl = slice(c * E_CORE, (c + 1) * E_CORE)
    en = elem_nodes[sl]                                   # [E, 8]
    disp = inputs[:, en]                                  # [B, E, 8, 2]
    u = disp[..., 0]                                      # [B, E, 8]
    v = disp[..., 1]
    w = u + v
    # D'[e, b, t, k] with t = (u, v, w)
    dstk = np.stack([u, v, w], axis=2)                    # [B, E, 3, 8]
    dstk = dstk.transpose(1, 0, 2, 3)                     # [E, B, 3, 8]
    dpad = np.zeros((E_PAD, D_EL), BF16)
    dpad[:E_CORE] = dstk.reshape(E_CORE, D_EL).astype(BF16)

    sx = shpdx[sl]                                        # [E, 9, 8, 2]
    s0 = sx[..., 0]
    s1 = sx[..., 1]
    a = 0.5 * (s0 + s1)
    sstk = np.stack([s0, s1, a], axis=2)                  # [E, 9, 3, 8]
    spad = np.zeros((E_PAD, S_EL), BF16)
    spad[:E_CORE] = sstk.reshape(E_CORE, S_EL).astype(BF16)
    return {"sp": spad, "dp": dpad}


def kernel(inputs, shpdx, elem_nodes, _want_trace=False):
    nc = _get_program()

    in_maps = [
        _marshal_core(inputs, shpdx, elem_nodes, c) for c in range(N_CORES)
    ]

    core_ids = list(range(N_CORES))
    res = run_bass_kernel_spmd(nc, in_maps, core_ids, trace=_want_trace)

    outs = []
    for c in range(N_CORES):
        o = res.results[c]["out"]                         # [B, E_PAD*9, 3] bf16
        outs.append(np.asarray(o[:, :E_CORE * N_GP, :], np.float32))
    full = np.concatenate(outs, axis=1)                   # [B, N_ELEM*9, 3]
    if _want_trace:
        return full, res
    return full


# revision 61
# speedup vs baseline: 1.0257x; 1.0213x over previous
"""Trainium2 kernel for the FEM kinematic (strain) layer.

Reference computation:
    disp = inputs[:, elem_nodes]                      # [B, E, 8, 2]
    dd   = einsum('egkl,bekn->begnl', shpdx, disp)    # [B, E, 9, 2, 2]
    out  = stack([dd[...,0,0], dd[...,1,1],
                  0.5*(dd[...,0,1] + dd[...,1,0])])   # [B, E*9, 3]

Sharding: elements split across 8 NeuronCores.  The host resolves the
element->node indirection and ships each core bf16 element-major blocks:
  S' = (s0, s1, (s0+s1)/2) per (e,g,k)   and   D' = (u, v, u+v) per (e,b,k).
The device computes, per (b,e,g), the three 8-term dot products
  P0 = s0.u, P1 = s1.v, P2 = 0.5*(s0+s1).(u+v)
via one bf16 tensor_tensor multiply (DVE 2x mode) + a pairwise add tree,
then combines  xx = P0, yy = P1, xy = P2 - 0.5*P0 - 0.5*P1.
Tree/combine work is split between the Vector and GpSimd engines.
"""

import sys
import numpy as np

sys.path.insert(0, "/opt/trn_rl_repo")

import concourse.bass as bass
import concourse.bacc as bacc
import concourse.mybir as mybir
import concourse.tile as tile
from concourse.bass_utils import run_bass_kernel_spmd

import ml_dtypes

BF16 = ml_dtypes.bfloat16

B = 4
N_NODES = 1_000_000
N_ELEM = 500_000
N_GP = 9
N_EN = 8
N_CORES = 8

E_CORE = N_ELEM // N_CORES            # 62500 elements per core
P = 128                               # SBUF partitions
C = 32                                # elements per partition per full chunk
IO_BUFS = 3
MID_BUFS = 1                          # per-b tile tags already pipeline
T_BUFS = 1
POOL_L1 = 0                           # L1 all on DVE; GpSimd takes L2+xy
XY_ACT = 1                            # xy fixup via ScalarE halves + GpSimd adds
DVE_L2 = 1                            # one L2 on DVE, rest GpSimd
CHUNK = P * C                         # 4864 elements per full chunk
N_FULL = E_CORE // CHUNK              # 12 full chunks
REM = E_CORE - N_FULL * CHUNK         # 4132 leftover elements
C_LAST = -(-REM // P)                 # 33: last-chunk elements/partition
N_CHUNKS = N_FULL + (1 if REM else 0)
E_PAD = N_FULL * CHUNK + (P * C_LAST if REM else 0)   # 62592
# small chunk first: shrinks the ramp-gating first S-DMA
CS = ([C_LAST] if REM else []) + [C] * N_FULL
OFFS = ([0] if REM else []) + [(P * C_LAST if REM else 0) + i * CHUNK
                               for i in range(N_FULL)]

S_EL = N_GP * 3 * N_EN                # 216: per element (g, t, k)
D_EL = B * 3 * N_EN                   # 96:  per element (b, t, k)
O_EL = B * N_GP * 3                   # 108: per element (b, g, t)

_compiled = None


def _build_program():
    nc = bacc.Bacc("TRN2", target_bir_lowering=False, debug=False)
    f32 = mybir.dt.float32
    bf16 = mybir.dt.bfloat16

    s_d = nc.dram_tensor("sp", [E_PAD, S_EL], bf16, kind="ExternalInput").ap()
    d_d = nc.dram_tensor("dp", [E_PAD, D_EL], bf16, kind="ExternalInput").ap()
    o_d = nc.dram_tensor("out", [B, E_PAD * N_GP, 3], bf16, kind="ExternalOutput").ap()

    def s_view(i):
        ci = CS[i]
        return s_d[OFFS[i]:OFFS[i] + P * ci].rearrange(
            "(p c) f -> p (c f)", p=P, c=ci)

    def d_view(i):
        ci = CS[i]
        return d_d[OFFS[i]:OFFS[i] + P * ci].rearrange(
            "(p c) f -> p (c f)", p=P, c=ci)

    def o_view(i):
        ci = CS[i]
        return o_d[:, OFFS[i] * N_GP:(OFFS[i] + P * ci) * N_GP].rearrange(
            "b (p c g) t -> p b (c g t)", p=P, c=ci, g=N_GP)

    with tile.TileContext(nc) as tc:
        with (
            tc.tile_pool(name="io", bufs=IO_BUFS) as io_pool,
            tc.tile_pool(name="mid", bufs=MID_BUFS) as mid_pool,
            tc.tile_pool(name="tp", bufs=T_BUFS) as t_pool,
            tc.tile_pool(name="u2p", bufs=2) as u2_pool,
        ):
            def issue_front(i):
                """DMAs + products + tree for chunk i; returns tail state."""
                ci = CS[i]
                S = io_pool.tile([P, C * S_EL], bf16, tag="S")
                D = io_pool.tile([P, C * D_EL], bf16, tag="D")
                nc.sync.dma_start(out=S[:, :ci * S_EL], in_=s_view(i))
                nc.sync.dma_start(out=D[:, :ci * D_EL], in_=d_view(i))

                Sv = S[:, :ci * S_EL].rearrange(
                    "p (c g t k) -> p c g t k", c=ci, g=N_GP, t=3)
                Dv = D[:, :ci * D_EL].rearrange(
                    "p (c b t k) -> p c b t k", c=ci, b=B, t=3)

                Tbs, U2bs = [], []
                U1bs = []
                def mk_prod(b):
                    Tb = t_pool.tile([P, C * 216], bf16, tag=f"T{b}")
                    Tbs.append(Tb)
                    db = Dv[:, :, b]
                    dbg = db[:, :, None, :, :].to_broadcast([P, ci, N_GP, 3, N_EN])
                    Tbv = Tb[:, :ci * 216].rearrange("p (c g t k) -> p c g t k", c=ci, g=N_GP, t=3)
                    nc.vector.tensor_tensor(
                        out=Tbv, in0=Sv, in1=dbg, op=mybir.AluOpType.mult
                    )
                def mk_l1(b):
                    U1b = mid_pool.tile([P, C * 108], bf16, tag=f"U1{b}")
                    U1bs.append(U1b)
                    Tbv = Tbs[b][:, :ci * 216].rearrange("p (c g t k) -> p c g t k", c=ci, g=N_GP, t=3)
                    U1bv = U1b[:, :ci * 108].rearrange("p (c g t k) -> p c g t k", c=ci, g=N_GP, t=3)
                    eng = nc.gpsimd if b < POOL_L1 else nc.vector
                    eng.tensor_tensor(
                        out=U1bv,
                        in0=Tbv[:, :, :, :, 0:4],
                        in1=Tbv[:, :, :, :, 4:8],
                        op=mybir.AluOpType.add,
                    )
                for b in range(B):
                    mk_prod(b)
                for b in range(B):
                    mk_l1(b)
                for b in range(B):
                    U2b = u2_pool.tile([P, C * 54], bf16, tag=f"U2{b}")
                    U2bs.append(U2b)
                    U1bv = U1bs[b][:, :ci * 108].rearrange("p (c g t k) -> p c g t k", c=ci, g=N_GP, t=3)
                    U2bv = U2b[:, :ci * 54].rearrange("p (c g t k) -> p c g t k", c=ci, g=N_GP, t=3)
                    eng2 = nc.vector if b < DVE_L2 else nc.gpsimd
                    eng2.tensor_tensor(
                        out=U2bv,
                        in0=U1bv[:, :, :, :, 0:2],
                        in1=U1bv[:, :, :, :, 2:4],
                        op=mybir.AluOpType.add,
                    )
                return ci, U2bs

            def issue_tail(i, ci, U2bs, last=False):
                """Strain combine + xy fixup + output DMA for chunk i."""
                O = io_pool.tile([P, B * C * 27], bf16, tag="O")
                Ocb = O[:, :B * ci * 27].rearrange(
                    "p (b c g t) -> p c b g t", b=B, c=ci, g=N_GP)
                TMP = u2_pool.tile([P, C * B * N_GP], bf16, tag="TMP")
                TMPv = TMP[:, :ci * B * N_GP].rearrange(
                    "p (c b g) -> p c b g", c=ci, b=B)
                if not last:
                    TMP2 = u2_pool.tile([P, C * B * N_GP], bf16, tag="TMP2")
                    TMP2v = TMP2[:, :ci * B * N_GP].rearrange(
                        "p (c b g) -> p c b g", c=ci, b=B)
                xx_o = Ocb[:, :, :, :, 0]
                yy_o = Ocb[:, :, :, :, 1]
                xy_o = Ocb[:, :, :, :, 2]
                for b in range(B):
                    U2bv = U2bs[b][:, :ci * 54].rearrange(
                        "p (c g t k) -> p c g t k", c=ci, g=N_GP, t=3
                    )
                    # one add writes (xx, yy, P2) straight into O's three
                    # strain slots; the xy slot is then fixed up in place:
                    #   xy = P2 - 0.5*xx - 0.5*yy
                    engc = nc.gpsimd
                    engc.tensor_tensor(
                        out=Ocb[:, :, b], in0=U2bv[:, :, :, :, 0],
                        in1=U2bv[:, :, :, :, 1], op=mybir.AluOpType.add,
                    )
                    if XY_ACT and not last:
                        # xy fixup without touching DVE: ScalarE halves,
                        # GpSimd adds
                        nc.scalar.activation(
                            out=TMPv[:, :, b], in_=xx_o[:, :, b],
                            func=mybir.ActivationFunctionType.Copy, scale=-0.5,
                        )
                        nc.scalar.activation(
                            out=TMP2v[:, :, b], in_=yy_o[:, :, b],
                            func=mybir.ActivationFunctionType.Copy, scale=-0.5,
                        )
                        engx = nc.gpsimd
                        engx.tensor_tensor(
                            out=TMPv[:, :, b], in0=TMPv[:, :, b],
                            in1=TMP2v[:, :, b], op=mybir.AluOpType.add,
                        )
                        engx.tensor_tensor(
                            out=xy_o[:, :, b], in0=TMPv[:, :, b],
                            in1=xy_o[:, :, b], op=mybir.AluOpType.add,
                        )
                    else:
                        # short serial chain for the final drain
                        nc.vector.scalar_tensor_tensor(
                            out=TMPv[:, :, b], in0=xx_o[:, :, b], scalar=-0.5,
                            in1=xy_o[:, :, b],
                            op0=mybir.AluOpType.mult, op1=mybir.AluOpType.add,
                        )
                        nc.vector.scalar_tensor_tensor(
                            out=xy_o[:, :, b], in0=yy_o[:, :, b], scalar=-0.5,
                            in1=TMPv[:, :, b],
                            op0=mybir.AluOpType.mult, op1=mybir.AluOpType.add,
                        )
                Ob = O[:, :B * ci * 27].rearrange("p (b f) -> p b f", b=B)
                # split per b-pair: first half ships while b2/b3 still combine
                ov = o_view(i)
                nc.sync.dma_start(out=ov[:, 0:2], in_=Ob[:, 0:2])
                nc.sync.dma_start(out=ov[:, 2:4], in_=Ob[:, 2:4])

            # software-pipelined issue: chunk i's tail is issued after chunk
            # i+1's front so the DVE FIFO never head-blocks on Pool's L2
            # software-pipelined issue: chunk i's tail (combine + xy fixup +
            # output DMA) is issued after chunk i+1's front, so the DVE FIFO
            # never head-blocks on GpSimd's L2 results
            # software-pipelined issue: chunk i's tail (combine + xy fixup +
            # output DMA) is issued after chunk i+1's front, so the DVE FIFO
            # never head-blocks on GpSimd's L2 results
            prev = None
            for i in range(N_CHUNKS):
                ci_u2 = issue_front(i)
                if prev is not None:
                    issue_tail(i - 1, *prev)
                prev = ci_u2
            issue_tail(N_CHUNKS - 1, *prev, last=True)

    nc.compile()
    return nc


def _get_program():
    global _compiled
    if _compiled is None:
        _compiled = _build_program()
    return _compiled


def _marshal_core(inputs, shpdx, elem_nodes, c):
    """Build the per-core bf16 S'/D' blocks."""
    sl = slice(c * E_CORE, (c + 1) * E_CORE)
    en = elem_nodes[sl]                                   # [E, 8]
    disp = inputs[:, en]                                  # [B, E, 8, 2]
    u = disp[..., 0]                                      # [B, E, 8]
    v = disp[..., 1]
    w = u + v
    # D'[e, b, t, k] with t = (u, v, w)
    dstk = np.stack([u, v, w], axis=2)                    # [B, E, 3, 8]
    dstk = dstk.transpose(1, 0, 2, 3)                     # [E, B, 3, 8]
    dpad = np.zeros((E_PAD, D_EL), BF16)
    dpad[:E_CORE] = dstk.reshape(E_CORE, D_EL).astype(BF16)

    sx = shpdx[sl]                                        # [E, 9, 8, 2]
    s0 = sx[..., 0]
    s1 = sx[..., 1]
    a = 0.5 * (s0 + s1)
    sstk = np.stack([s0, s1, a], axis=2)                  # [E, 9, 3, 8]
    spad = np.zeros((E_PAD, S_EL), BF16)
    spad[:E_CORE] = sstk.reshape(E_CORE, S_EL).astype(BF16)
    return {"sp": spad, "dp": dpad}


def kernel(inputs, shpdx, elem_nodes, _want_trace=False):
    nc = _get_program()

    in_maps = [
        _marshal_core(inputs, shpdx, elem_nodes, c) for c in range(N_CORES)
    ]

    core_ids = list(range(N_CORES))
    res = run_bass_kernel_spmd(nc, in_maps, core_ids, trace=_want_trace)

    outs = []
    for c in range(N_CORES):
        o = res.results[c]["out"]                         # [B, E_PAD*9, 3] bf16
        outs.append(np.asarray(o[:, :E_CORE * N_GP, :], np.float32))
    full = np.concatenate(outs, axis=1)                   # [B, N_ELEM*9, 3]
    if _want_trace:
        return full, res
    return full
